# revision 1
# baseline (speedup 1.0000x reference)
"""MultiHeadedAttention Trainium2 Bass kernel.

Reference (per batch element b, full shapes B=8, S=1024, D=512, H=8, DK=64):
    Q = x_q @ Wq + bq ; K = x_k @ Wk + bk ; V = x_v @ Wv + bv   (per-head split)
    S = Q K^T / sqrt(DK);  S masked where mask==0 -> -inf
    P = softmax(S); P zeroed where mask==0
    Y = (P V, heads concat) @ Wo + bo

Sharding: pure data parallel over batch — core c computes batch element c.
No collectives. Host transposes x inputs so the kernel needs no on-chip
input transposes, and precomputes the additive exp-space mask bias.

Per-core layout (f32 in HBM; matmuls run as f32r, PSUM accumulates f32):
  xT        [in=512, S]  (host-transposed; DMA interleaved with weights so
                          the first projection starts after ~3MB, not 12MB)
  QT, KT    [feat, S]   psum[out128, q512] += Wq[in128, out128].T @ xT[in128, q512]
  V natural [S, feat]   psum[row128, f512] += xT_v[in128, row128].T @ Wv[in128, f512]
                        stored interleaved as v_aug[row128, head, 65] with a
                        ones column per head (softmax denominator for free);
                        one strided DVE copy per row tile
  S^T       [k128, q512] = KT_h[d64, k128].T @ QT_h[d64, q512]
                        head pairs packed into PE row groups 0/64 via
                        tile_position -> both matmuls run concurrently
  P^T       = Exp(S^T/8 + maskbias_k)      (ACT, one call per [128,1024])
  (PV)^T+den[65, q512]  += v_aug_h[k128, 65].T @ P^T[k128, q512]  (row 64 = denom)
  norm      at_pair[t][h%2*64 :+64, q] = (PV)^T * bcast(1/denom)
            (DVE cross-base-partition write packs head pairs -> K=128 below)
  Y natural [q128, 512] += at_pair[t][:, q128].T @ Wo[feat128, out512] (+ bo)
"""

import numpy as np

B, S, D, H = 8, 1024, 512, 8
DK = D // H  # 64
P = 128
KI = D // P  # 4 in-feature tiles
RT = S // P  # 8 row tiles
QC = S // 512  # 2 q chunks of 512
HP = H // 2  # 4 head pairs
MASK_NEG = -30000.0  # exp(-30000) == 0.0 in f32

_CACHED = {}


def _build_nc(loop_reps=None):
    import concourse.mybir as mybir
    import concourse.tile as tile
    from concourse import bacc

    f32 = mybir.dt.float32
    f32r = mybir.dt.float32r
    EXP = mybir.ActivationFunctionType.Exp
    ISCALE = 1.0 / float(np.sqrt(DK))

    nc = bacc.Bacc("TRN2")

    xqT_d = nc.dram_tensor("xqT", (KI, P, S), f32r, kind="ExternalInput")
    xkT_d = nc.dram_tensor("xkT", (KI, P, S), f32r, kind="ExternalInput")
    xvT_d = nc.dram_tensor("xvT", (KI, P, S), f32r, kind="ExternalInput")
    maskb_d = nc.dram_tensor("maskb", (P, RT), f32, kind="ExternalInput")
    wq_d = nc.dram_tensor("wq", (KI, P, D), f32r, kind="ExternalInput")
    wk_d = nc.dram_tensor("wk", (KI, P, D), f32r, kind="ExternalInput")
    wv_d = nc.dram_tensor("wv", (KI, P, D), f32r, kind="ExternalInput")
    wo_d = nc.dram_tensor("wo", (KI, P, D), f32r, kind="ExternalInput")
    bq_d = nc.dram_tensor("bq", (P, KI), f32, kind="ExternalInput")
    bk_d = nc.dram_tensor("bk", (P, KI), f32, kind="ExternalInput")
    bv_d = nc.dram_tensor("bv", (1, D), f32r, kind="ExternalInput")
    bo_d = nc.dram_tensor("bo", (1, D), f32r, kind="ExternalInput")
    y_d = nc.dram_tensor("y", (RT, P, D), f32, kind="ExternalOutput")

    with tile.TileContext(nc) as tc, nc.allow_low_precision(
        reason="f32r is fp32-width storage; matmul accumulation stays fp32 in PSUM"
    ):
        from contextlib import ExitStack

        def emit():
            with ExitStack() as ctx:
                const = ctx.enter_context(tc.tile_pool(name="const", bufs=1))
                persist = ctx.enter_context(tc.tile_pool(name="persist", bufs=1))

                wq = [const.tile([P, D], f32r, name=f"wq{i}", tag=f"wq{i}") for i in range(KI)]
                wk = [const.tile([P, D], f32r, name=f"wk{i}", tag=f"wk{i}") for i in range(KI)]
                wv = [const.tile([P, D], f32r, name=f"wv{i}", tag=f"wv{i}") for i in range(KI)]
                wo = [const.tile([P, D], f32r, name=f"wo{i}", tag=f"wo{i}") for i in range(KI)]
                bq_t = const.tile([P, KI], f32, name="bq_t", tag="bq")
                bk_t = const.tile([P, KI], f32, name="bk_t", tag="bk")
                bv_t = const.tile([1, D], f32r, name="bv_t", tag="bv")
                bo_t = const.tile([1, D], f32r, name="bo_t", tag="bo")
                maskb = const.tile([P, RT], f32, name="maskb", tag="maskb")
                ones_t = const.tile([P, P], f32r, name="ones_t", tag="ones")
                nc.vector.memset(ones_t[:].bitcast(f32), 1.0)

                # persistent intermediates
                qt = [persist.tile([P, S], f32r, name=f"qt{i}", tag=f"qt{i}") for i in range(KI)]
                kt_ = [persist.tile([P, S], f32r, name=f"kt{i}", tag=f"kt{i}") for i in range(KI)]
                v_aug = [persist.tile([P, H, DK + 1], f32r, name=f"va{i}", tag=f"va{i}") for i in range(RT)]
                # head-pair attention outputs: pair t rows 0:64 = head 2t,
                # rows 64:128 = head 2t+1 => feature rows 128t..128t+127
                at = [persist.tile([P, S], f32r, name=f"at{i}", tag=f"at{i}") for i in range(HP)]

                with ExitStack() as actx:
                    xt_pool = actx.enter_context(tc.tile_pool(name="xt", bufs=1))
                    psA = actx.enter_context(
                        tc.tile_pool(name="psA", bufs=4, space="PSUM")
                    )
                    xqT = [xt_pool.tile([P, S], f32r, name=f"xq{i}", tag=f"xq{i}") for i in range(KI)]
                    xkT = [xt_pool.tile([P, S], f32r, name=f"xk{i}", tag=f"xk{i}") for i in range(KI)]
                    xvT = [xt_pool.tile([P, S], f32r, name=f"xv{i}", tag=f"xv{i}") for i in range(KI)]

                    # DMA in consumption order (queue is FIFO): q-path first
                    # so the first projection can start after ~3MB.
                    # Two HWDGE queues: q-path then v-path on sync's queue,
                    # k-path then output weights on ScalarE's queue (descriptor
                    # issue only; ACT is idle during the load). QT and KT
                    # stream in parallel so the first S^T starts ~2x sooner.
                    for i in range(KI):
                        nc.sync.dma_start(wq[i][:], wq_d[i])
                        nc.sync.dma_start(xqT[i][:], xqT_d[i])
                        nc.scalar.dma_start(wk[i][:], wk_d[i])
                        nc.scalar.dma_start(xkT[i][:], xkT_d[i])
                    nc.sync.dma_start(bq_t[:], bq_d[:])
                    nc.scalar.dma_start(bk_t[:], bk_d[:])
                    nc.scalar.dma_start(maskb[:], maskb_d[:])
                    for i in range(KI):
                        nc.sync.dma_start(wv[i][:], wv_d[i])
                        nc.sync.dma_start(xvT[i][:], xvT_d[i])
                    nc.sync.dma_start(bv_t[:], bv_d[:])
                    for i in range(KI):
                        nc.scalar.dma_start(wo[i][:], wo_d[i])
                    nc.scalar.dma_start(bo_t[:], bo_d[:])

                    # QT / KT projections
                    for w, x, bias, dst in ((wq, xqT, bq_t, qt), (wk, xkT, bk_t, kt_)):
                        for o in range(KI):
                            for qc in range(QC):
                                ps = psA.tile([P, 512], f32, name="psA", tag="psA")
                                for ki in range(KI):
                                    nc.tensor.matmul(
                                        ps[:],
                                        w[ki][:, o * P : (o + 1) * P],
                                        x[ki][:, qc * 512 : (qc + 1) * 512],
                                        start=(ki == 0),
                                        stop=(ki == KI - 1),
                                    )
                                nc.vector.tensor_scalar_add(
                                    dst[o][:, qc * 512 : (qc + 1) * 512],
                                    ps[:],
                                    bias[:, o : o + 1],
                                )

                    # V natural -> v_aug (interleaved heads + ones columns)
                    for rt in range(RT):
                        ps = psA.tile([P, 512], f32, name="psA", tag="psA")
                        for ki in range(KI):
                            nc.tensor.matmul(
                                ps[:],
                                xvT[ki][:, rt * P : (rt + 1) * P],
                                wv[ki][:],
                                start=(ki == 0),
                                stop=False,
                            )
                        nc.tensor.matmul(
                            ps[:],
                            ones_t[0:1, 0:P],
                            bv_t[0:1, :],
                            start=False,
                            stop=True,
                        )
                        nc.vector.tensor_copy(
                            v_aug[rt][:, :, 0:DK],
                            ps[:].rearrange("p (h d) -> p h d", h=H),
                        )
                        nc.vector.memset(
                            v_aug[rt][:, :, DK : DK + 1].bitcast(f32), 1.0
                        )

                # --- attention, one head pair at a time ---
                with ExitStack() as bctx:
                    pt_pool = bctx.enter_context(tc.tile_pool(name="pt", bufs=21))
                    rec_pool = bctx.enter_context(tc.tile_pool(name="rec", bufs=4))
                    at_ps = bctx.enter_context(
                        tc.tile_pool(name="spsum", bufs=2, space="PSUM")
                    )
                    ov_ps = bctx.enter_context(
                        tc.tile_pool(name="opsum", bufs=3, space="PSUM")
                    )
                    rb_ps = bctx.enter_context(
                        tc.tile_pool(name="rbpsum", bufs=1, space="PSUM")
                    )

                    for t in range(HP):
                        pts = [
                            [pt_pool.tile([P, S], f32r, name="pt", tag="pt") for _ in range(RT)]
                            for _ in range(2)
                        ]
                        # sub 0's PV chains consume pt tiles in lockstep with
                        # the exp stream so half the pool frees at pair end
                        # (the next pair's exps then aren't slot-starved).
                        ops00 = ov_ps.tile([P, 512], f32, name="ops", tag="ops")
                        ops01 = ov_ps.tile([P, 512], f32, name="ops", tag="ops")
                        for kt in range(RT):
                            for sub in range(2):
                                off = sub * DK
                                sps = at_ps.tile([P, S], f32, name="sps", tag="sps")
                                for qc in range(QC):
                                    nc.tensor.matmul(
                                        sps[:, qc * 512 : (qc + 1) * 512],
                                        kt_[t][off : off + DK, kt * P : (kt + 1) * P],
                                        qt[t][off : off + DK, qc * 512 : (qc + 1) * 512],
                                        start=True,
                                        stop=True,
                                        tile_position=(off, 0),
                                    )
                                nc.scalar.activation(
                                    pts[sub][kt][:],
                                    sps[:],
                                    EXP,
                                    bias=maskb[:, kt : kt + 1],
                                    scale=ISCALE,
                                )
                            for qc, ops in ((0, ops00), (1, ops01)):
                                nc.tensor.matmul(
                                    ops[0 : DK + 1, :],
                                    v_aug[kt][:, 2 * t, 0 : DK + 1],
                                    pts[0][kt][:, qc * 512 : (qc + 1) * 512],
                                    start=(kt == 0),
                                    stop=(kt == RT - 1),
                                )
                        for sub in range(2):
                            h = 2 * t + sub
                            off = sub * DK
                            for qc in range(QC):
                                if sub == 0:
                                    ops = ops00 if qc == 0 else ops01
                                else:
                                    ops = ov_ps.tile(
                                        [P, 512], f32, name="ops", tag="ops"
                                    )
                                    for kt in range(RT):
                                        nc.tensor.matmul(
                                            ops[0 : DK + 1, :],
                                            v_aug[kt][:, h, 0 : DK + 1],
                                            pts[sub][kt][:, qc * 512 : (qc + 1) * 512],
                                            start=(kt == 0),
                                            stop=(kt == RT - 1),
                                        )
                                rec = rec_pool.tile(
                                    [DK + 1, 512], f32r, name="rec", tag="rec"
                                )
                                nc.vector.reciprocal(
                                    rec[DK : DK + 1, :], ops[DK : DK + 1, :]
                                )
                                rb = rb_ps.tile([P, 512], f32, name="rb", tag="rb")
                                nc.tensor.matmul(
                                    rb[0:DK, :],
                                    ones_t[DK : DK + 1, 0:DK],
                                    rec[DK : DK + 1, :],
                                    start=True,
                                    stop=True,
                                )
                                # DVE has one PSUM read port: stage broadcast
                                # reciprocal in SBUF, then multiply (write may
                                # shift base partition by 64 for odd heads).
                                rbs = rec_pool.tile(
                                    [DK, 512], f32, name="rbs", tag="rbs"
                                )
                                nc.vector.tensor_copy(rbs[:], rb[0:DK, :])
                                nc.vector.tensor_mul(
                                    at[t][off : off + DK, qc * 512 : (qc + 1) * 512],
                                    ops[0:DK, :],
                                    rbs[:],
                                )

                # --- output projection: contraction K=128 over head pairs ---
                with ExitStack() as cctx:
                    y_pool = cctx.enter_context(tc.tile_pool(name="y", bufs=3))
                    y_ps = cctx.enter_context(
                        tc.tile_pool(name="ypsum", bufs=2, space="PSUM")
                    )
                    for rt in range(RT):
                        yps = y_ps.tile([P, D], f32, name="yps", tag="yps")
                        for t in range(HP):
                            nc.tensor.matmul(
                                yps[:],
                                at[t][:, rt * P : (rt + 1) * P],
                                wo[t][:],
                                start=(t == 0),
                                stop=False,
                            )
                        nc.tensor.matmul(
                            yps[:],
                            ones_t[0:1, 0:P],
                            bo_t[0:1, :],
                            start=False,
                            stop=True,
                        )
                        yt = y_pool.tile([P, D], f32, name="yt", tag="yt")
                        nc.vector.tensor_copy(yt[:], yps[:])
                        nc.sync.dma_start(y_d[rt], yt[:])

        if loop_reps is None:
            emit()
        else:
            # benchmark variant: repeat the whole body on-device
            ET = mybir.EngineType
            with tc.For_i(
                0,
                loop_reps,
                1,
                hint_engines=(ET.PE, ET.Activation, ET.DVE, ET.SP, ET.Pool),
            ):
                emit()

    nc.compile()
    return nc


def get_nc(loop_reps=None):
    key = ("nc", loop_reps)
    if key not in _CACHED:
        _CACHED[key] = _build_nc(loop_reps)
    return _CACHED[key]


def make_in_maps(query, key, value, mask, Wq, bq, Wk, bk, Wv, bv, Wo, bo):
    """Shard full inputs into per-core input maps (host-side numpy)."""
    f = np.float32
    query = np.asarray(query, f)
    key = np.asarray(key, f)
    value = np.asarray(value, f)
    mask = np.asarray(mask)

    def wtiles(W):
        return np.ascontiguousarray(np.asarray(W, f).reshape(KI, P, D))

    wq_t, wk_t, wv_t, wo_t = wtiles(Wq), wtiles(Wk), wtiles(Wv), wtiles(Wo)
    bq_t = np.ascontiguousarray(np.asarray(bq, f).reshape(KI, P).T)
    bk_t = np.ascontiguousarray(np.asarray(bk, f).reshape(KI, P).T)
    bv_t = np.ascontiguousarray(np.asarray(bv, f).reshape(1, D))
    bo_t = np.ascontiguousarray(np.asarray(bo, f).reshape(1, D))

    in_maps = []
    for c in range(B):
        xqT = np.ascontiguousarray(query[c].T).reshape(KI, P, S)
        xkT = np.ascontiguousarray(key[c].T).reshape(KI, P, S)
        xvT = np.ascontiguousarray(value[c].T).reshape(KI, P, S)
        mb = np.where(mask[c, 0] == 0, f(MASK_NEG), f(0.0)).astype(f)
        mb = np.ascontiguousarray(mb.reshape(RT, P).T)
        in_maps.append(
            {
                "xqT": xqT,
                "xkT": xkT,
                "xvT": xvT,
                "maskb": mb,
                "wq": wq_t,
                "wk": wk_t,
                "wv": wv_t,
                "wo": wo_t,
                "bq": bq_t,
                "bk": bk_t,
                "bv": bv_t,
                "bo": bo_t,
            }
        )
    return in_maps


def kernel(**inputs):
    from concourse.bass_utils import run_bass_kernel_spmd

    nc = get_nc()
    in_maps = make_in_maps(**inputs)
    res = run_bass_kernel_spmd(nc, in_maps, core_ids=list(range(B)))
    out = np.stack([res.results[c]["y"].reshape(S, D) for c in range(B)])
    return out.astype(np.float32)



# revision 11
# speedup vs baseline: 2.7152x; 2.7152x over previous
"""MultiHeadedAttention Trainium2 Bass kernel (v2: bf16 + KV compaction).

Reference (per batch element b, full shapes B=8, S=1024, D=512, H=8, DK=64):
    Q = x_q @ Wq + bq ; K = x_k @ Wk + bk ; V = x_v @ Wv + bv   (per-head split)
    S = Q K^T / sqrt(DK);  S masked where mask==0 -> -inf
    P = softmax(S); P zeroed where mask==0
    Y = (P V, heads concat) @ Wo + bo

Sharding: pure data parallel over batch — core c computes batch element c.
No collectives.

Host-side prep: transposes x inputs, casts everything to bf16, and
COMPACTS the key/value sequences by dropping masked-out positions
(attention is permutation/drop invariant over masked keys: their probs
are exactly 0). Kept positions are padded to SK (multiple of 128, 640
for the seed-0 inputs) with maskbias -30000 so exp gives exactly 0.
This cuts scores/exp/PV work by ~SK/S.

Per-core layout (bf16 operands, PSUM accumulates f32):
  xqT       [in=512, S]   host-transposed
  xkT, xvT  [in=512, SK]  host-compacted + transposed
  QT        [feat, S]   psum[out128, q512] += Wq[in128, out128].T @ xqT[in128, q512]
                        DVE tensor_scalar_add drains psum + adds bias + casts bf16
  KT        [feat, SK]  same with Wk/xkT
  V natural [SK, feat]  psum[row128, f512] += xvT[in128, row128].T @ Wv[in128, f512]
                        (+ ones-row matmul adds bv) stored as v_aug[row128, h, 65]
                        with a ones column per head (softmax denominator for free)
  S^T       [k128, q512] = KT_h[d64, k128].T @ QT_h[d64, q512]
                        head pairs packed into PE row groups 0/64 via tile_position;
                        at bf16 the two 64-row streams run concurrently at 2 rows/cyc
  P^T       = Exp(S^T/8 + maskbias_k)   (ACT, one call per [128,1024], bf16 out)
  (PV)^T+den[65, q512]  += v_aug_h[k128, 65].T @ P^T[k128, q512]  (row 64 = denom)
  norm      at[t][h%2*64 :+64, q] = (PV)^T * bcast(1/denom)  (pair-packed broadcast)
  Y natural [q128, 512] += at[t][:, q128].T @ Wo[feat128, out512] (+ bo ones-row)
"""

import numpy as np

B, S, D, H = 8, 1024, 512, 8
DK = D // H  # 64
P = 128
KI = D // P  # 4 in-feature tiles
RT = S // P  # 8 q row tiles
QC = S // 512  # 2 q chunks of 512
HP = H // 2  # 4 head pairs
SK_DEFAULT = 640
MASK_NEG = -30000.0  # exp(-30000) == 0.0 in f32

_CACHED = {}


def _build_nc(loop_reps=None, sk=SK_DEFAULT):
    import concourse.mybir as mybir
    import concourse.tile as tile
    from concourse import bacc

    f32 = mybir.dt.float32
    f32r = mybir.dt.float32r
    bf16 = mybir.dt.bfloat16
    EXP = mybir.ActivationFunctionType.Exp
    ISCALE = 1.0 / float(np.sqrt(DK))
    KC = sk // P  # kv chunks of 128

    nc = bacc.Bacc("TRN2")

    xqT_d = nc.dram_tensor("xqT", (KI, P, S), bf16, kind="ExternalInput")
    xkT_d = nc.dram_tensor("xkT", (KI, P, sk), bf16, kind="ExternalInput")
    xvT_d = nc.dram_tensor("xvT", (KI, P, sk), bf16, kind="ExternalInput")
    maskb_d = nc.dram_tensor("maskb", (P, KC), f32, kind="ExternalInput")
    wq_d = nc.dram_tensor("wq", (KI, P, D), bf16, kind="ExternalInput")
    wk_d = nc.dram_tensor("wk", (KI, P, D), bf16, kind="ExternalInput")
    wv_d = nc.dram_tensor("wv", (KI, P, D), bf16, kind="ExternalInput")
    wo_d = nc.dram_tensor("wo", (KI, P, D), bf16, kind="ExternalInput")
    bq_d = nc.dram_tensor("bq", (P, KI), f32, kind="ExternalInput")
    bk_d = nc.dram_tensor("bk", (P, KI), f32, kind="ExternalInput")
    bv_d = nc.dram_tensor("bv", (1, D), bf16, kind="ExternalInput")
    bo_d = nc.dram_tensor("bo", (1, D), bf16, kind="ExternalInput")
    y_d = nc.dram_tensor("y", (RT, P, D), f32, kind="ExternalOutput")

    with tile.TileContext(nc) as tc, nc.allow_low_precision(
        reason="bf16 matmuls with fp32 PSUM accumulation; tolerance is 2e-2"
    ):
        from contextlib import ExitStack

        def emit():
            with ExitStack() as ctx:
                const = ctx.enter_context(tc.tile_pool(name="const", bufs=1))
                persist = ctx.enter_context(tc.tile_pool(name="persist", bufs=1))

                wq = [const.tile([P, D], bf16, name=f"wq{i}", tag=f"wq{i}") for i in range(KI)]
                wk = [const.tile([P, D], bf16, name=f"wk{i}", tag=f"wk{i}") for i in range(KI)]
                wv = [const.tile([P, D], bf16, name=f"wv{i}", tag=f"wv{i}") for i in range(KI)]
                wo = [const.tile([P, D], bf16, name=f"wo{i}", tag=f"wo{i}") for i in range(KI)]
                bq_t = const.tile([P, KI], f32, name="bq_t", tag="bq")
                bk_t = const.tile([P, KI], f32, name="bk_t", tag="bk")
                bv_t = const.tile([1, D], bf16, name="bv_t", tag="bv")
                bo_t = const.tile([1, D], bf16, name="bo_t", tag="bo")
                maskb = const.tile([P, KC], f32, name="maskb", tag="maskb")
                ones_t = const.tile([P, P], bf16, name="ones_t", tag="ones")
                nc.vector.memset(ones_t[:].bitcast(bf16), 1.0)
                # expander rows: row 0 = [1]*64+[0]*64 (partition 0), row 32 =
                # [0]*64+[1]*64. Two accumulating 1-row matmuls broadcast the
                # two reciprocal rows into rb[0:64]/rb[64:128] in one psum.
                exp_t = const.tile([33, P], f32r, name="exp_t", tag="expt")
                nc.vector.memset(exp_t[0:1, :].bitcast(f32), 0.0)
                nc.vector.memset(exp_t[32:33, :].bitcast(f32), 0.0)
                nc.vector.memset(exp_t[0:1, 0:DK].bitcast(f32), 1.0)
                nc.vector.memset(exp_t[32:33, DK:P].bitcast(f32), 1.0)

                # persistent intermediates
                qt = [persist.tile([P, S], bf16, name=f"qt{i}", tag=f"qt{i}") for i in range(KI)]
                kt_ = [persist.tile([P, sk], bf16, name=f"kt{i}", tag=f"kt{i}") for i in range(KI)]
                v_aug = [persist.tile([P, H, DK + 1], bf16, name=f"va{i}", tag=f"va{i}") for i in range(KC)]
                # head-pair attention outputs: pair t rows 0:64 = head 2t,
                # rows 64:128 = head 2t+1 => feature rows 128t..128t+127
                at = [persist.tile([P, S], bf16, name=f"at{i}", tag=f"at{i}") for i in range(HP)]

                with ExitStack() as actx:
                    xt_pool = actx.enter_context(tc.tile_pool(name="xt", bufs=1))
                    psA = actx.enter_context(
                        tc.tile_pool(name="psA", bufs=4, space="PSUM")
                    )
                    xqT = [xt_pool.tile([P, S], bf16, name=f"xq{i}", tag=f"xq{i}") for i in range(KI)]
                    xkT = [xt_pool.tile([P, sk], bf16, name=f"xk{i}", tag=f"xk{i}") for i in range(KI)]
                    xvT = [xt_pool.tile([P, sk], bf16, name=f"xv{i}", tag=f"xv{i}") for i in range(KI)]

                    # DMA in consumption order (queues are FIFO): q+k first so
                    # the first projections and scores start ASAP. Keep the
                    # ACT engine's queue free — exp is the bottleneck engine.
                    for i in range(KI):
                        nc.sync.dma_start(wq[i][:], wq_d[i])
                        nc.sync.dma_start(xqT[i][:], xqT_d[i])
                        nc.gpsimd.dma_start(wk[i][:], wk_d[i])
                        nc.gpsimd.dma_start(xkT[i][:], xkT_d[i])
                    nc.sync.dma_start(bq_t[:], bq_d[:])
                    nc.gpsimd.dma_start(bk_t[:], bk_d[:])
                    nc.gpsimd.dma_start(maskb[:], maskb_d[:])
                    for i in range(KI):
                        nc.sync.dma_start(wv[i][:], wv_d[i])
                        nc.sync.dma_start(xvT[i][:], xvT_d[i])
                    nc.sync.dma_start(bv_t[:], bv_d[:])
                    for i in range(KI):
                        nc.gpsimd.dma_start(wo[i][:], wo_d[i])
                    nc.gpsimd.dma_start(bo_t[:], bo_d[:])

                    # QT / KT projections; DVE drain adds bias + casts bf16.
                    # Emit per o-tile pair (Q then K) so attention pair 0 can
                    # start as soon as o-tile 0 of both is done.
                    for o in range(KI):
                        for w, x, bias, dst, n in (
                            (wq, xqT, bq_t, qt, S),
                            (wk, xkT, bk_t, kt_, sk),
                        ):
                            for qc0 in range(0, n, 512):
                                qw = min(512, n - qc0)
                                ps = psA.tile([P, 512], f32, name="psA", tag="psA")
                                for ki in range(KI):
                                    nc.tensor.matmul(
                                        ps[:, 0:qw],
                                        w[ki][:, o * P : (o + 1) * P],
                                        x[ki][:, qc0 : qc0 + qw],
                                        start=(ki == 0),
                                        stop=(ki == KI - 1),
                                    )
                                nc.vector.tensor_scalar_add(
                                    dst[o][:, qc0 : qc0 + qw],
                                    ps[:, 0:qw],
                                    bias[:, o : o + 1],
                                )

                    # V natural -> v_aug (interleaved heads + ones columns)
                    for rt in range(KC):
                        ps = psA.tile([P, 512], f32, name="psA", tag="psA")
                        for ki in range(KI):
                            nc.tensor.matmul(
                                ps[:],
                                xvT[ki][:, rt * P : (rt + 1) * P],
                                wv[ki][:],
                                start=(ki == 0),
                                stop=False,
                            )
                        nc.tensor.matmul(
                            ps[:],
                            ones_t[0:1, 0:P],
                            bv_t[0:1, :],
                            start=False,
                            stop=True,
                        )
                        nc.vector.tensor_copy(
                            v_aug[rt][:, :, 0:DK],
                            ps[:].rearrange("p (h d) -> p h d", h=H),
                        )
                        nc.vector.memset(
                            v_aug[rt][:, :, DK : DK + 1].bitcast(bf16), 1.0
                        )

                # --- attention, one head pair at a time ---
                with ExitStack() as bctx:
                    pt_pool = bctx.enter_context(tc.tile_pool(name="pt", bufs=14))
                    rec_pool = bctx.enter_context(tc.tile_pool(name="rec", bufs=4))
                    at_ps = bctx.enter_context(
                        tc.tile_pool(name="spsum", bufs=2, space="PSUM")
                    )
                    ov_ps = bctx.enter_context(
                        tc.tile_pool(name="opsum", bufs=3, space="PSUM")
                    )
                    rb_ps = bctx.enter_context(
                        tc.tile_pool(name="rbpsum", bufs=1, space="PSUM")
                    )

                    for t in range(HP):
                        pts = [
                            [pt_pool.tile([P, S], bf16, name="pt", tag="pt") for _ in range(KC)]
                            for _ in range(2)
                        ]
                        # sub 0's PV chains consume pt tiles in lockstep with
                        # the exp stream so half the pool frees at pair end.
                        ops00 = ov_ps.tile([P, 512], f32, name="ops", tag="ops")
                        ops01 = ov_ps.tile([P, 512], f32, name="ops", tag="ops")
                        for kt in range(KC):
                            spss = [
                                at_ps.tile([P, S], f32, name="sps", tag="sps")
                                for _ in range(2)
                            ]
                            # interleave the two 64-row subs so the PE row
                            # groups 0/64 stream concurrently
                            for qc in range(QC):
                                for sub in range(2):
                                    off = sub * DK
                                    nc.tensor.matmul(
                                        spss[sub][:, qc * 512 : (qc + 1) * 512],
                                        kt_[t][off : off + DK, kt * P : (kt + 1) * P],
                                        qt[t][off : off + DK, qc * 512 : (qc + 1) * 512],
                                        start=True,
                                        stop=True,
                                        tile_position=(off, 0),
                                    )
                            for sub in range(2):
                                nc.scalar.activation(
                                    pts[sub][kt][:],
                                    spss[sub][:],
                                    EXP,
                                    bias=maskb[:, kt : kt + 1],
                                    scale=ISCALE,
                                )
                            for qc, ops in ((0, ops00), (1, ops01)):
                                nc.tensor.matmul(
                                    ops[0 : DK + 1, :],
                                    v_aug[kt][:, 2 * t, 0 : DK + 1],
                                    pts[0][kt][:, qc * 512 : (qc + 1) * 512],
                                    start=(kt == 0),
                                    stop=(kt == KC - 1),
                                )
                        for qc in range(QC):
                            opss = [ops00 if qc == 0 else ops01, None]
                            ops1 = ov_ps.tile([P, 512], f32, name="ops", tag="ops")
                            opss[1] = ops1
                            for kt in range(KC):
                                nc.tensor.matmul(
                                    ops1[0 : DK + 1, :],
                                    v_aug[kt][:, 2 * t + 1, 0 : DK + 1],
                                    pts[1][kt][:, qc * 512 : (qc + 1) * 512],
                                    start=(kt == 0),
                                    stop=(kt == KC - 1),
                                )
                            # pair-packed normalization: reciprocal each sub's
                            # denominator row into adjacent partitions, then one
                            # expander matmul broadcasts both into rb [128,512].
                            rec = rec_pool.tile(
                                [33, 512], f32r, name="rec", tag="rec"
                            )
                            rb = rb_ps.tile([P, 512], f32, name="rb", tag="rb")
                            for sub in range(2):
                                nc.vector.reciprocal(
                                    rec[32 * sub : 32 * sub + 1, :],
                                    opss[sub][DK : DK + 1, :],
                                )
                                nc.tensor.matmul(
                                    rb[:],
                                    exp_t[32 * sub : 32 * sub + 1, :],
                                    rec[32 * sub : 32 * sub + 1, :],
                                    start=(sub == 0),
                                    stop=(sub == 1),
                                    tile_position=(32 * sub, 0),
                                )
                            # DVE has one PSUM read port: stage broadcast
                            # reciprocal in SBUF, then multiply per sub (each
                            # sub's PV lives in its own psum tile).
                            rbs = rec_pool.tile(
                                [P, 512], f32, name="rbs", tag="rbs"
                            )
                            nc.vector.tensor_copy(rbs[:], rb[:])
                            for sub in range(2):
                                off = sub * DK
                                nc.vector.tensor_mul(
                                    at[t][off : off + DK, qc * 512 : (qc + 1) * 512],
                                    opss[sub][0:DK, :],
                                    rbs[64 * sub : 64 * sub + DK, :],
                                )

                # --- output projection: contraction K=128 over head pairs ---
                with ExitStack() as cctx:
                    y_pool = cctx.enter_context(tc.tile_pool(name="y", bufs=3))
                    y_ps = cctx.enter_context(
                        tc.tile_pool(name="ypsum", bufs=2, space="PSUM")
                    )
                    for rt in range(RT):
                        yps = y_ps.tile([P, D], f32, name="yps", tag="yps")
                        for t in range(HP):
                            nc.tensor.matmul(
                                yps[:],
                                at[t][:, rt * P : (rt + 1) * P],
                                wo[t][:],
                                start=(t == 0),
                                stop=False,
                            )
                        nc.tensor.matmul(
                            yps[:],
                            ones_t[0:1, 0:P],
                            bo_t[0:1, :],
                            start=False,
                            stop=True,
                        )
                        yt = y_pool.tile([P, D], f32, name="yt", tag="yt")
                        nc.vector.tensor_copy(yt[:], yps[:])
                        nc.sync.dma_start(y_d[rt], yt[:])

        if loop_reps is None:
            emit()
        else:
            # benchmark variant: repeat the whole body on-device
            ET = mybir.EngineType
            with tc.For_i(
                0,
                loop_reps,
                1,
                hint_engines=(ET.PE, ET.Activation, ET.DVE, ET.SP, ET.Pool),
            ):
                emit()

    nc.compile()
    return nc


def get_nc(loop_reps=None, sk=SK_DEFAULT):
    key = ("nc", loop_reps, sk)
    if key not in _CACHED:
        _CACHED[key] = _build_nc(loop_reps, sk)
    return _CACHED[key]


def _sk_for_mask(mask):
    counts = np.asarray(mask).reshape(B, S).astype(np.int64).sum(axis=1)
    need = max(int(counts.max()), 1)
    return max(P, int(np.ceil(need / P) * P))


def make_in_maps(query, key, value, mask, Wq, bq, Wk, bk, Wv, bv, Wo, bo):
    """Shard full inputs into per-core input maps (host-side numpy)."""
    import ml_dtypes

    bf = ml_dtypes.bfloat16
    f = np.float32
    query = np.asarray(query, f)
    key = np.asarray(key, f)
    value = np.asarray(value, f)
    mask = np.asarray(mask)
    sk = _sk_for_mask(mask)
    kc = sk // P

    def wtiles(W):
        return np.ascontiguousarray(np.asarray(W, f).reshape(KI, P, D)).astype(bf)

    wq_t, wk_t, wv_t, wo_t = wtiles(Wq), wtiles(Wk), wtiles(Wv), wtiles(Wo)
    bq_t = np.ascontiguousarray(np.asarray(bq, f).reshape(KI, P).T)
    bk_t = np.ascontiguousarray(np.asarray(bk, f).reshape(KI, P).T)
    bv_t = np.ascontiguousarray(np.asarray(bv, f).reshape(1, D)).astype(bf)
    bo_t = np.ascontiguousarray(np.asarray(bo, f).reshape(1, D)).astype(bf)

    in_maps = []
    for c in range(B):
        keep = np.flatnonzero(mask[c, 0] != 0)
        nk = len(keep)
        kc_pad = np.zeros((sk, D), f)
        kc_pad[:nk] = key[c][keep]
        vc_pad = np.zeros((sk, D), f)
        vc_pad[:nk] = value[c][keep]
        mb = np.full(sk, f(MASK_NEG), f)
        mb[:nk] = 0.0
        mb = np.ascontiguousarray(mb.reshape(kc, P).T)

        xqT = np.ascontiguousarray(query[c].T).reshape(KI, P, S).astype(bf)
        xkT = np.ascontiguousarray(kc_pad.T).reshape(KI, P, sk).astype(bf)
        xvT = np.ascontiguousarray(vc_pad.T).reshape(KI, P, sk).astype(bf)
        in_maps.append(
            {
                "xqT": xqT,
                "xkT": xkT,
                "xvT": xvT,
                "maskb": mb,
                "wq": wq_t,
                "wk": wk_t,
                "wv": wv_t,
                "wo": wo_t,
                "bq": bq_t,
                "bk": bk_t,
                "bv": bv_t,
                "bo": bo_t,
            }
        )
    return in_maps, sk


def kernel(**inputs):
    from concourse.bass_utils import run_bass_kernel_spmd

    in_maps, sk = make_in_maps(**inputs)
    nc = get_nc(sk=sk)
    res = run_bass_kernel_spmd(nc, in_maps, core_ids=list(range(B)))
    out = np.stack([res.results[c]["y"].reshape(S, D) for c in range(B)])
    return out.astype(np.float32)


# revision 23
# speedup vs baseline: 2.9493x; 1.0862x over previous
"""MultiHeadedAttention Trainium2 Bass kernel (v3: software-pipelined).

Reference (per batch element b, full shapes B=8, S=1024, D=512, H=8, DK=64):
    Q = x_q @ Wq + bq ; K = x_k @ Wk + bk ; V = x_v @ Wv + bv   (per-head split)
    S = Q K^T / sqrt(DK);  S masked where mask==0 -> -inf
    P = softmax(S); P zeroed where mask==0
    Y = (P V, heads concat) @ Wo + bo

Sharding: pure data parallel over batch — core c computes batch element c.

Host-side prep: transposes x inputs, casts to bf16, and COMPACTS the
key/value sequences by dropping masked-out positions (their probs are
exactly 0), padding to SK (multiple of 128; 640 for the seed-0 inputs)
with maskbias -30000 so exp underflows to exactly 0.

The ACT engine's exp stream (one [128,1024] Exp per kv-chunk per head,
~1.04us each) is the bottleneck; everything else is emitted so ACT never
starves:
  - scores for pair t feed exp directly (64-row head-pair matmuls run
    concurrently in PE row groups 0/64 at 2 bf16 rows/cycle);
  - pair t's PV chains + normalization are DEFERRED and dripped between
    pair t+1's score matmuls (one drip unit per kv-chunk slot);
  - Q/K/V projection chains for later pairs drip the same way;
  - biases ride existing psum-drain ops (tensor_scalar_add for Q/K,
    broadcast tensor_tensor add for V and the output).
PSUM: scores 2x[128,1024](4 banks) + PV 2 + recip-broadcast 1 + proj 1.
"""

import numpy as np

B, S, D, H = 8, 1024, 512, 8
DK = D // H  # 64
P = 128
KI = D // P  # 4 in-feature tiles
RT = S // P  # 8 q row tiles
QC = S // 512  # 2 q chunks of 512
HP = H // 2  # 4 head pairs
SK_DEFAULT = 640
MASK_NEG = -30000.0  # exp(-30000) == 0.0 in f32

_CACHED = {}


def _build_nc(loop_reps=None, sk=SK_DEFAULT):
    import concourse.mybir as mybir
    import concourse.tile as tile
    from concourse import bacc

    f32 = mybir.dt.float32
    f32r = mybir.dt.float32r
    bf16 = mybir.dt.bfloat16
    EXP = mybir.ActivationFunctionType.Exp
    ISCALE = 1.0 / float(np.sqrt(DK))
    KC = sk // P  # kv chunks of 128

    nc = bacc.Bacc("TRN2")

    xqT_d = nc.dram_tensor("xqT", (KI, P, S), bf16, kind="ExternalInput")
    xkT_d = nc.dram_tensor("xkT", (KI, P, sk), bf16, kind="ExternalInput")
    xvT_d = nc.dram_tensor("xvT", (KI, P, sk), bf16, kind="ExternalInput")
    maskb_d = nc.dram_tensor("maskb", (P, KC), f32, kind="ExternalInput")
    wq_d = nc.dram_tensor("wq", (KI, P, D), bf16, kind="ExternalInput")
    wk_d = nc.dram_tensor("wk", (KI, P, D), bf16, kind="ExternalInput")
    wv_d = nc.dram_tensor("wv", (KI, P, D), bf16, kind="ExternalInput")
    wo_d = nc.dram_tensor("wo", (KI, P, D), bf16, kind="ExternalInput")
    bq_d = nc.dram_tensor("bq", (P, KI), f32, kind="ExternalInput")
    bk_d = nc.dram_tensor("bk", (P, KI), f32, kind="ExternalInput")
    bv_d = nc.dram_tensor("bv", (1, D), bf16, kind="ExternalInput")
    bo_d = nc.dram_tensor("bo", (1, D), bf16, kind="ExternalInput")
    y_d = nc.dram_tensor("y", (RT, P, D), f32, kind="ExternalOutput")

    with tile.TileContext(nc) as tc, nc.allow_low_precision(
        reason="bf16 matmuls with fp32 PSUM accumulation; tolerance is 2e-2"
    ):
        from contextlib import ExitStack

        def emit():
            with ExitStack() as ctx:
                const = ctx.enter_context(tc.tile_pool(name="const", bufs=1))
                persist = ctx.enter_context(tc.tile_pool(name="persist", bufs=1))
                xt_pool = ctx.enter_context(tc.tile_pool(name="xt", bufs=1))
                pt_pool = ctx.enter_context(tc.tile_pool(name="pt", bufs=24))
                rec_pool = ctx.enter_context(tc.tile_pool(name="rec", bufs=4))
                attn_ctx = ctx.enter_context(ExitStack())
                ov_ps = attn_ctx.enter_context(
                    tc.tile_pool(name="opsum", bufs=2, space="PSUM")
                )
                # shared rotating [128,512] psum pool: projection chains AND
                # reciprocal broadcasts (mixed-tag FIFO rotation, depth 2)
                psA = attn_ctx.enter_context(
                    tc.tile_pool(name="psA", bufs=2, space="PSUM")
                )
                rb_ps = psA
                # scores psum opens LAST so it can close first (LIFO) to
                # free its 4 banks for the output projection
                sps_ctx = ctx.enter_context(ExitStack())
                at_ps = sps_ctx.enter_context(
                    tc.tile_pool(name="spsum", bufs=2, space="PSUM")
                )

                wq = [const.tile([P, D], bf16, name=f"wq{i}", tag=f"wq{i}") for i in range(KI)]
                wk = [const.tile([P, D], bf16, name=f"wk{i}", tag=f"wk{i}") for i in range(KI)]
                wv = [const.tile([P, D], bf16, name=f"wv{i}", tag=f"wv{i}") for i in range(KI)]
                wo = [const.tile([P, D], bf16, name=f"wo{i}", tag=f"wo{i}") for i in range(KI)]
                bq_t = const.tile([P, KI], f32, name="bq_t", tag="bq")
                bk_t = const.tile([P, KI], f32, name="bk_t", tag="bk")
                bv_t = const.tile([1, D], bf16, name="bv_t", tag="bv")
                bo_t = const.tile([1, D], bf16, name="bo_t", tag="bo")
                maskb = const.tile([P, KC], f32, name="maskb", tag="maskb")
                ones_t = const.tile([1, P], bf16, name="ones_t", tag="ones")
                nc.vector.memset(ones_t[:].bitcast(bf16), 1.0)
                # expander rows for the pair-packed reciprocal broadcast:
                # row 0 = [1]*64+[0]*64, row 32 = [0]*64+[1]*64
                exp_t = const.tile([33, P], f32r, name="exp_t", tag="expt")
                nc.vector.memset(exp_t[0:1, :].bitcast(f32), 0.0)
                nc.vector.memset(exp_t[32:33, :].bitcast(f32), 0.0)
                nc.vector.memset(exp_t[0:1, 0:DK].bitcast(f32), 1.0)
                nc.vector.memset(exp_t[32:33, DK:P].bitcast(f32), 1.0)
                # broadcast biases to all 128 partitions (for psum-drain adds)
                bv_b = const.tile([P, D], f32, name="bv_b", tag="bvb")
                bo_b = const.tile([P, D], f32, name="bo_b", tag="bob")

                # persistent intermediates
                qt = [persist.tile([P, S], bf16, name=f"qt{i}", tag=f"qt{i}") for i in range(KI)]
                kt_ = [persist.tile([P, sk], bf16, name=f"kt{i}", tag=f"kt{i}") for i in range(KI)]
                v_aug = [persist.tile([P, H, DK + 1], bf16, name=f"va{i}", tag=f"va{i}") for i in range(KC)]
                at = [persist.tile([P, S], bf16, name=f"at{i}", tag=f"at{i}") for i in range(HP)]

                xqT = [xt_pool.tile([P, S], bf16, name=f"xq{i}", tag=f"xq{i}") for i in range(KI)]
                xkT = [xt_pool.tile([P, sk], bf16, name=f"xk{i}", tag=f"xk{i}") for i in range(KI)]
                xvT = [xt_pool.tile([P, sk], bf16, name=f"xv{i}", tag=f"xv{i}") for i in range(KI)]

                # DMA in consumption order (queues are FIFO), spread over four
                # engines' queues to cut the startup fill. ACT's queue is
                # never used: exp is the bottleneck engine.
                # scalar (ACT) queue only for early loads — it idles ~6us
                # before the first exp; descriptor issue fits in that window
                for i in range(2):
                    nc.sync.dma_start(wq[i][:], wq_d[i])
                    nc.sync.dma_start(xqT[i][:], xqT_d[i])
                    nc.scalar.dma_start(wq[i + 2][:], wq_d[i + 2])
                    nc.scalar.dma_start(xqT[i + 2][:], xqT_d[i + 2])
                    nc.gpsimd.dma_start(wk[i][:], wk_d[i])
                    nc.gpsimd.dma_start(xkT[i][:], xkT_d[i])
                nc.scalar.dma_start(wk[2][:], wk_d[2])
                nc.scalar.dma_start(xkT[2][:], xkT_d[2])
                nc.gpsimd.dma_start(wk[3][:], wk_d[3])
                nc.gpsimd.dma_start(xkT[3][:], xkT_d[3])
                nc.sync.dma_start(bq_t[:], bq_d[:])
                nc.gpsimd.dma_start(bk_t[:], bk_d[:])
                nc.gpsimd.dma_start(maskb[:], maskb_d[:])
                for i in range(KI):
                    nc.sync.dma_start(wv[i][:], wv_d[i])
                    nc.sync.dma_start(xvT[i][:], xvT_d[i])
                nc.sync.dma_start(bv_t[:], bv_d[:])
                for i in range(KI):
                    nc.gpsimd.dma_start(wo[i][:], wo_d[i])
                nc.gpsimd.dma_start(bo_t[:], bo_d[:])

                # materialize bias broadcasts via ones-column matmuls
                def bias_broadcast(dst, src):
                    ps = psA.tile([P, 512], f32, name="psA", tag="psA")
                    nc.tensor.matmul(
                        ps[:], ones_t[0:1, :], src[0:1, :], start=True, stop=True
                    )
                    nc.vector.tensor_copy(dst[:], ps[:])

                # ---- drip task generators (each yields small PE units) ----

                def qk_chain(w, x, bias, dst, o, c0, cw):
                    ps = psA.tile([P, 512], f32, name="psA", tag="psA")
                    for ki in range(KI):
                        nc.tensor.matmul(
                            ps[:, 0:cw],
                            w[ki][:, o * P : (o + 1) * P],
                            x[ki][:, c0 : c0 + cw],
                            start=(ki == 0),
                            stop=(ki == KI - 1),
                        )
                    nc.vector.tensor_scalar_add(
                        dst[o][:, c0 : c0 + cw], ps[:, 0:cw], bias[:, o : o + 1]
                    )

                def v_chain(rt):
                    ps = psA.tile([P, 512], f32, name="psA", tag="psA")
                    for ki in range(KI):
                        nc.tensor.matmul(
                            ps[:],
                            xvT[ki][:, rt * P : (rt + 1) * P],
                            wv[ki][:],
                            start=(ki == 0),
                            stop=(ki == KI - 1),
                        )
                    nc.vector.tensor_add(
                        v_aug[rt][:, :, 0:DK],
                        ps[:].rearrange("p (h d) -> p h d", h=H),
                        bv_b[:].rearrange("p (h d) -> p h d", h=H),
                    )
                    nc.vector.memset(v_aug[rt][:, :, DK : DK + 1].bitcast(bf16), 1.0)

                def pv_and_norm_tasks(t, pts):
                    """Deferred PV + normalization for pair t, as drip units."""
                    for qc in range(QC):
                        opss = []
                        for sub in range(2):
                            ops = ov_ps.tile([P, 512], f32, name="ops", tag="ops")
                            opss.append(ops)

                            def pv(ops=ops, sub=sub, qc=qc):
                                for kt in range(KC):
                                    nc.tensor.matmul(
                                        ops[0 : DK + 1, :],
                                        v_aug[kt][:, 2 * t + sub, 0 : DK + 1],
                                        pts[sub][kt][:, qc * 512 : (qc + 1) * 512],
                                        start=(kt == 0),
                                        stop=(kt == KC - 1),
                                    )

                            yield pv

                        def norm(opss=opss, qc=qc):
                            rec = rec_pool.tile([33, 512], f32r, name="rec", tag="rec")
                            rb = rb_ps.tile([P, 512], f32, name="rb", tag="psA")
                            for sub in range(2):
                                nc.vector.reciprocal(
                                    rec[32 * sub : 32 * sub + 1, :],
                                    opss[sub][DK : DK + 1, :],
                                )
                                nc.tensor.matmul(
                                    rb[:],
                                    exp_t[32 * sub : 32 * sub + 1, :],
                                    rec[32 * sub : 32 * sub + 1, :],
                                    start=(sub == 0),
                                    stop=(sub == 1),
                                    tile_position=(32 * sub, 0),
                                )
                            rbs = rec_pool.tile([P, 512], f32, name="rbs", tag="rbs")
                            nc.vector.tensor_copy(rbs[:], rb[:])
                            for sub in range(2):
                                off = sub * DK
                                nc.vector.tensor_mul(
                                    at[t][off : off + DK, qc * 512 : (qc + 1) * 512],
                                    opss[sub][0:DK, :],
                                    rbs[64 * sub : 64 * sub + DK, :],
                                )

                        yield norm

                def scores_window(t, drip):
                    """Emit pair t's scores+exp; between kv-chunks, pop drip
                    tasks (prev pair's PV/norm, later projections)."""
                    pts = [
                        [pt_pool.tile([P, S], bf16, name="pt", tag="pt") for _ in range(KC)]
                        for _ in range(2)
                    ]
                    for kt in range(KC):
                        spss = [
                            at_ps.tile([P, S], f32, name="sps", tag="sps")
                            for _ in range(2)
                        ]
                        for qc in range(QC):
                            for sub in range(2):
                                off = sub * DK
                                nc.tensor.matmul(
                                    spss[sub][:, qc * 512 : (qc + 1) * 512],
                                    kt_[t][off : off + DK, kt * P : (kt + 1) * P],
                                    qt[t][off : off + DK, qc * 512 : (qc + 1) * 512],
                                    start=True,
                                    stop=True,
                                    tile_position=(off, 0),
                                )
                        for sub in range(2):
                            nc.scalar.activation(
                                pts[sub][kt][:],
                                spss[sub][:],
                                EXP,
                                bias=maskb[:, kt : kt + 1],
                                scale=ISCALE,
                            )
                        if drip:
                            drip.pop(0)()
                        if drip:
                            drip.pop(0)()
                    return pts

                # ---- schedule ----
                kchunks = [(c0, min(512, sk - c0)) for c0 in range(0, sk, 512)]

                def proj_tasks(o):
                    ts = []
                    for c0 in range(0, S, 512):
                        ts.append(lambda o=o, c0=c0: qk_chain(wq, xqT, bq_t, qt, o, c0, 512))
                    for c0, cw in kchunks:
                        ts.append(lambda o=o, c0=c0, cw=cw: qk_chain(wk, xkT, bk_t, kt_, o, c0, cw))
                    return ts

                # pair 0's projections run up front; interleave Q/K chunks so
                # scores kt=0 unblocks as early as possible
                t0 = proj_tasks(0)
                for task in (t0[0], t0[1], t0[2], t0[3]):
                    task()

                # drip queues per window: V chains + later projections join
                # the previous pair's deferred PV/norm units
                drip = [lambda: bias_broadcast(bv_b, bv_t)]
                drip += [lambda rt=rt: v_chain(rt) for rt in range(KC)]
                drip += proj_tasks(1)
                drip += [lambda: bias_broadcast(bo_b, bo_t)]
                pts_prev = scores_window(0, drip)
                for task in drip:  # leftovers
                    task()

                for t in range(1, HP):
                    drip = list(pv_and_norm_tasks(t - 1, pts_prev))
                    if t + 1 < HP:
                        drip += proj_tasks(t + 1)
                    pts_prev = scores_window(t, drip)
                    for task in drip:  # leftovers (shouldn't happen)
                        task()

                # tail: release scores banks, interleave pair 3's deferred
                # PV/norm with the output projection (rt 0-3 need only at
                # columns 0:512 = qc 0; rt 4-7 need qc 1)
                sps_ctx.close()
                with ExitStack() as cctx:
                    y_pool = cctx.enter_context(tc.tile_pool(name="y", bufs=3))
                    y_ps = cctx.enter_context(
                        tc.tile_pool(name="ypsum", bufs=2, space="PSUM")
                    )

                    def yo_chain(rt):
                        yps = y_ps.tile([P, D], f32, name="yps", tag="yps")
                        for t in range(HP):
                            nc.tensor.matmul(
                                yps[:],
                                at[t][:, rt * P : (rt + 1) * P],
                                wo[t][:],
                                start=(t == 0),
                                stop=(t == HP - 1),
                            )
                        yt = y_pool.tile([P, D], f32, name="yt", tag="yt")
                        nc.vector.tensor_add(yt[:], yps[:], bo_b[:])
                        nc.sync.dma_start(y_d[rt], yt[:])

                    tail = list(pv_and_norm_tasks(HP - 1, pts_prev))
                    # [pv,pv,norm] for qc0, then rt0-3, then qc1, then rt4-7
                    for task in tail[0:3]:
                        task()
                    for rt in range(0, 2):
                        yo_chain(rt)
                    for task in tail[3:5]:
                        task()
                    for rt in range(2, 4):
                        yo_chain(rt)
                    tail[5]()
                    for rt in range(4, RT):
                        yo_chain(rt)

        if loop_reps is None:
            emit()
        else:
            ET = mybir.EngineType
            with tc.For_i(
                0,
                loop_reps,
                1,
                hint_engines=(ET.PE, ET.Activation, ET.DVE, ET.SP, ET.Pool),
            ):
                emit()

    nc.compile()
    return nc


def get_nc(loop_reps=None, sk=SK_DEFAULT):
    key = ("nc", loop_reps, sk)
    if key not in _CACHED:
        _CACHED[key] = _build_nc(loop_reps, sk)
    return _CACHED[key]


def _sk_for_mask(mask):
    counts = np.asarray(mask).reshape(B, S).astype(np.int64).sum(axis=1)
    need = max(int(counts.max()), 1)
    return max(P, int(np.ceil(need / P) * P))


def make_in_maps(query, key, value, mask, Wq, bq, Wk, bk, Wv, bv, Wo, bo):
    """Shard full inputs into per-core input maps (host-side numpy)."""
    import ml_dtypes

    bf = ml_dtypes.bfloat16
    f = np.float32
    query = np.asarray(query, f)
    key = np.asarray(key, f)
    value = np.asarray(value, f)
    mask = np.asarray(mask)
    sk = _sk_for_mask(mask)
    kc = sk // P

    def wtiles(W):
        return np.ascontiguousarray(np.asarray(W, f).reshape(KI, P, D)).astype(bf)

    wq_t, wk_t, wv_t, wo_t = wtiles(Wq), wtiles(Wk), wtiles(Wv), wtiles(Wo)
    bq_t = np.ascontiguousarray(np.asarray(bq, f).reshape(KI, P).T)
    bk_t = np.ascontiguousarray(np.asarray(bk, f).reshape(KI, P).T)
    bv_t = np.ascontiguousarray(np.asarray(bv, f).reshape(1, D)).astype(bf)
    bo_t = np.ascontiguousarray(np.asarray(bo, f).reshape(1, D)).astype(bf)

    in_maps = []
    for c in range(B):
        keep = np.flatnonzero(mask[c, 0] != 0)
        nk = len(keep)
        kc_pad = np.zeros((sk, D), f)
        kc_pad[:nk] = key[c][keep]
        vc_pad = np.zeros((sk, D), f)
        vc_pad[:nk] = value[c][keep]
        mb = np.full(sk, f(MASK_NEG), f)
        mb[:nk] = 0.0
        mb = np.ascontiguousarray(mb.reshape(kc, P).T)

        xqT = np.ascontiguousarray(query[c].T).reshape(KI, P, S).astype(bf)
        xkT = np.ascontiguousarray(kc_pad.T).reshape(KI, P, sk).astype(bf)
        xvT = np.ascontiguousarray(vc_pad.T).reshape(KI, P, sk).astype(bf)
        in_maps.append(
            {
                "xqT": xqT,
                "xkT": xkT,
                "xvT": xvT,
                "maskb": mb,
                "wq": wq_t,
                "wk": wk_t,
                "wv": wv_t,
                "wo": wo_t,
                "bq": bq_t,
                "bk": bk_t,
                "bv": bv_t,
                "bo": bo_t,
            }
        )
    return in_maps, sk


def kernel(**inputs):
    from concourse.bass_utils import run_bass_kernel_spmd

    in_maps, sk = make_in_maps(**inputs)
    nc = get_nc(sk=sk)
    res = run_bass_kernel_spmd(nc, in_maps, core_ids=list(range(B)))
    out = np.stack([res.results[c]["y"].reshape(S, D) for c in range(B)])
    return out.astype(np.float32)


# revision 28
# speedup vs baseline: 3.2319x; 1.0958x over previous
"""MultiHeadedAttention Trainium2 Bass kernel (v4: deep pipelining).

Reference (per batch element b, full shapes B=8, S=1024, D=512, H=8, DK=64):
    Q = x_q @ Wq + bq ; K = x_k @ Wk + bk ; V = x_v @ Wv + bv   (per-head split)
    S = Q K^T / sqrt(DK);  S masked where mask==0 -> -inf
    P = softmax(S); P zeroed where mask==0
    Y = (P V, heads concat) @ Wo + bo

Sharding: pure data parallel over batch — core c computes batch element c.

Host prep: transpose x inputs, cast to bf16, COMPACT keys/values by the
mask (masked keys have exactly-zero probs; attention is drop-invariant),
pad to SK=ceil(max_count/128)*128 with maskbias -30000 (exp -> 0).

Engine plan (per core, SK=640):
  ACT: 40x Exp[128,1024] ~42us — the bottleneck stream; never used for DMA.
  PE:  projections (4-chains), 64-row head-pair score matmuls (row groups
       0/64 run concurrently at 2 bf16 rows/cyc), PV 65-row chains (ones
       column in v_aug gives the softmax denominator for free), out-proj.
  DVE: psum drains (+bias, bf16 cast), reciprocals, normalization muls.
  Pool(gpsimd): DMA queue + partition_broadcast of reciprocal rows.
  Scheduling: pair t's PV/norm and pair t+1's projections DRIP between
  pair t+1's score matmuls so ACT never starves; input/weight/persist
  tiles are double-buffered so iteration i+1's DMA and projections overlap
  iteration i's attention tail (the For_i loop pipelines across
  iterations — engines sync only on data).
PSUM: scores 2x[128,1024] (4 banks) + shared [128,512] work pool x4
      (PV accumulators, projection chains, out-proj) = 8 banks.
"""

import numpy as np

B, S, D, H = 8, 1024, 512, 8
DK = D // H  # 64
P = 128
KI = D // P  # 4 in-feature tiles
RT = S // P  # 8 q row tiles
QC = S // 512  # 2 q chunks of 512
HP = H // 2  # 4 head pairs
SK_DEFAULT = 640
MASK_NEG = -30000.0  # exp(-30000) == 0.0 in f32

_CACHED = {}


def _build_nc(loop_reps=None, sk=SK_DEFAULT):
    import concourse.mybir as mybir
    import concourse.tile as tile
    from concourse import bacc

    f32 = mybir.dt.float32
    f32r = mybir.dt.float32r
    bf16 = mybir.dt.bfloat16
    EXP = mybir.ActivationFunctionType.Exp
    ISCALE = 1.0 / float(np.sqrt(DK))
    KC = sk // P  # kv chunks of 128

    nc = bacc.Bacc("TRN2")

    xqT_d = nc.dram_tensor("xqT", (KI, P, S), bf16, kind="ExternalInput")
    xkT_d = nc.dram_tensor("xkT", (KI, P, sk), bf16, kind="ExternalInput")
    xvT_d = nc.dram_tensor("xvT", (KI, P, sk), bf16, kind="ExternalInput")
    maskb_d = nc.dram_tensor("maskb", (P, KC), f32, kind="ExternalInput")
    wq_d = nc.dram_tensor("wq", (KI, P, D), bf16, kind="ExternalInput")
    wk_d = nc.dram_tensor("wk", (KI, P, D), bf16, kind="ExternalInput")
    wv_d = nc.dram_tensor("wv", (KI, P, D), bf16, kind="ExternalInput")
    wo_d = nc.dram_tensor("wo", (KI, P, D), bf16, kind="ExternalInput")
    bq_d = nc.dram_tensor("bq", (P, KI), f32, kind="ExternalInput")
    bk_d = nc.dram_tensor("bk", (P, KI), f32, kind="ExternalInput")
    bv_d = nc.dram_tensor("bv", (1, D), bf16, kind="ExternalInput")
    bo_d = nc.dram_tensor("bo", (1, D), bf16, kind="ExternalInput")
    y_d = nc.dram_tensor("y", (RT, P, D), f32, kind="ExternalOutput")

    with tile.TileContext(nc) as tc, nc.allow_low_precision(
        reason="bf16 matmuls with fp32 PSUM accumulation; tolerance is 2e-2"
    ):
        from contextlib import ExitStack

        def emit():
            with ExitStack() as ctx:
                # bufs=2 on inputs/persist: iteration i+1's DMA + projections
                # overlap iteration i's attention tail
                const = ctx.enter_context(tc.tile_pool(name="const", bufs=2))
                persist = ctx.enter_context(tc.tile_pool(name="persist", bufs=2))
                one_p = ctx.enter_context(tc.tile_pool(name="onep", bufs=1))
                pt_pool = ctx.enter_context(tc.tile_pool(name="pt", bufs=20))
                rec_pool = ctx.enter_context(tc.tile_pool(name="rec", bufs=2))
                work_ps = ctx.enter_context(
                    tc.tile_pool(name="work", bufs=4, space="PSUM")
                )
                sps_ctx = ctx.enter_context(ExitStack())
                at_ps = sps_ctx.enter_context(
                    tc.tile_pool(name="spsum", bufs=2, space="PSUM")
                )

                wq = [const.tile([P, D], bf16, name=f"wq{i}", tag=f"wq{i}") for i in range(KI)]
                wk = [const.tile([P, D], bf16, name=f"wk{i}", tag=f"wk{i}") for i in range(KI)]
                wv = [const.tile([P, D], bf16, name=f"wv{i}", tag=f"wv{i}") for i in range(KI)]
                wo = [const.tile([P, D], bf16, name=f"wo{i}", tag=f"wo{i}") for i in range(KI)]
                bq_t = const.tile([P, KI], f32, name="bq_t", tag="bq")
                bk_t = const.tile([P, KI], f32, name="bk_t", tag="bk")
                bv_t = const.tile([1, D], bf16, name="bv_t", tag="bv")
                bo_t = const.tile([1, D], bf16, name="bo_t", tag="bo")
                maskb = const.tile([P, KC], f32, name="maskb", tag="maskb")
                xqT = [const.tile([P, S], bf16, name=f"xq{i}", tag=f"xq{i}") for i in range(KI)]
                xkT = [const.tile([P, sk], bf16, name=f"xk{i}", tag=f"xk{i}") for i in range(KI)]
                xvT = [const.tile([P, sk], bf16, name=f"xv{i}", tag=f"xv{i}") for i in range(KI)]

                ones_t = one_p.tile([1, P], bf16, name="ones_t", tag="ones")
                nc.vector.memset(ones_t[:].bitcast(bf16), 1.0)
                # bias broadcast rows (built once per iteration via matmul)
                bv_b = persist.tile([P, D], f32, name="bv_b", tag="bvb")
                bo_b = persist.tile([P, D], f32, name="bo_b", tag="bob")

                qt = [persist.tile([P, S], bf16, name=f"qt{i}", tag=f"qt{i}") for i in range(KI)]
                kt_ = [persist.tile([P, sk], bf16, name=f"kt{i}", tag=f"kt{i}") for i in range(KI)]
                v_aug = [persist.tile([P, H, DK + 1], bf16, name=f"va{i}", tag=f"va{i}") for i in range(KC)]
                at = [persist.tile([P, S], bf16, name=f"at{i}", tag=f"at{i}") for i in range(HP)]

                # DMA: consumption order, balanced across the three HWDGE
                # queues (SP / gpsimd / ACT — ACT only issues before its exp
                # stream starts). y output DMA rides SP at the end.
                for i in range(2):
                    nc.sync.dma_start(wq[i][:], wq_d[i])
                    nc.sync.dma_start(xqT[i][:], xqT_d[i])
                    nc.scalar.dma_start(wq[i + 2][:], wq_d[i + 2])
                    nc.scalar.dma_start(xqT[i + 2][:], xqT_d[i + 2])
                    nc.gpsimd.dma_start(wk[i][:], wk_d[i])
                    nc.gpsimd.dma_start(xkT[i][:], xkT_d[i])
                nc.scalar.dma_start(wk[2][:], wk_d[2])
                nc.scalar.dma_start(xkT[2][:], xkT_d[2])
                nc.gpsimd.dma_start(wk[3][:], wk_d[3])
                nc.gpsimd.dma_start(xkT[3][:], xkT_d[3])
                nc.sync.dma_start(bq_t[:], bq_d[:])
                nc.gpsimd.dma_start(bk_t[:], bk_d[:])
                nc.gpsimd.dma_start(maskb[:], maskb_d[:])
                for i in range(2):
                    nc.scalar.dma_start(wv[i][:], wv_d[i])
                    nc.scalar.dma_start(xvT[i][:], xvT_d[i])
                    nc.gpsimd.dma_start(wv[i + 2][:], wv_d[i + 2])
                    nc.gpsimd.dma_start(xvT[i + 2][:], xvT_d[i + 2])
                nc.scalar.dma_start(bv_t[:], bv_d[:])
                for i in range(KI):
                    nc.gpsimd.dma_start(wo[i][:], wo_d[i])
                nc.gpsimd.dma_start(bo_t[:], bo_d[:])

                def bias_broadcast(dst, src):
                    ps = work_ps.tile([P, 512], f32, name="wps", tag="wps")
                    nc.tensor.matmul(
                        ps[:], ones_t[0:1, :], src[0:1, :], start=True, stop=True
                    )
                    nc.vector.tensor_copy(dst[:], ps[:])

                def qk_chain(w, x, bias, dst, o, c0, cw):
                    ps = work_ps.tile([P, 512], f32, name="wps", tag="wps")
                    for ki in range(KI):
                        nc.tensor.matmul(
                            ps[:, 0:cw],
                            w[ki][:, o * P : (o + 1) * P],
                            x[ki][:, c0 : c0 + cw],
                            start=(ki == 0),
                            stop=(ki == KI - 1),
                        )
                    nc.vector.tensor_scalar_add(
                        dst[o][:, c0 : c0 + cw], ps[:, 0:cw], bias[:, o : o + 1]
                    )

                def v_chain(rt):
                    ps = work_ps.tile([P, 512], f32, name="wps", tag="wps")
                    for ki in range(KI):
                        nc.tensor.matmul(
                            ps[:],
                            xvT[ki][:, rt * P : (rt + 1) * P],
                            wv[ki][:],
                            start=(ki == 0),
                            stop=(ki == KI - 1),
                        )
                    nc.vector.tensor_add(
                        v_aug[rt][:, :, 0:DK],
                        ps[:].rearrange("p (h d) -> p h d", h=H),
                        bv_b[:].rearrange("p (h d) -> p h d", h=H),
                    )
                    nc.vector.memset(v_aug[rt][:, :, DK : DK + 1].bitcast(bf16), 1.0)

                def pv_and_norm_tasks(t, pts):
                    """Deferred PV + normalization for pair t as drip units.
                    All four PV accumulators are independent (work pool);
                    normalization hangs off DVE/Pool without gating PE."""
                    for qc in range(QC):
                        opss = []
                        for sub in range(2):
                            ops = work_ps.tile([P, 512], f32, name="wps", tag="wps")
                            opss.append(ops)

                            def pv(ops=ops, sub=sub, qc=qc):
                                for kt in range(KC):
                                    nc.tensor.matmul(
                                        ops[0 : DK + 1, :],
                                        v_aug[kt][:, 2 * t + sub, 0 : DK + 1],
                                        pts[sub][kt][:, qc * 512 : (qc + 1) * 512],
                                        start=(kt == 0),
                                        stop=(kt == KC - 1),
                                    )

                            yield pv

                        def norm(opss=opss, qc=qc):
                            # HW partition_broadcast only writes/reads from
                            # partition base 0 — separate tiles per sub
                            for sub in range(2):
                                rec = rec_pool.tile(
                                    [1, 512], f32, name="rec", tag=f"rec{sub}"
                                )
                                rbs = rec_pool.tile(
                                    [DK, 512], f32, name="rbs", tag=f"rbs{sub}"
                                )
                                nc.vector.reciprocal(
                                    rec[0:1, :], opss[sub][DK : DK + 1, :]
                                )
                                nc.gpsimd.partition_broadcast(
                                    rbs[:], rec[0:1, :]
                                )
                                off = sub * DK
                                nc.vector.tensor_mul(
                                    at[t][off : off + DK, qc * 512 : (qc + 1) * 512],
                                    opss[sub][0:DK, :],
                                    rbs[:],
                                )

                        yield norm

                def scores_window(t, drip):
                    """Pair t's scores+exp; drip deferred tasks between
                    kv-chunks so PE work fills ACT's exp time."""
                    pts = [
                        [pt_pool.tile([P, S], bf16, name="pt", tag="pt") for _ in range(KC)]
                        for _ in range(2)
                    ]
                    for kt in range(KC):
                        spss = [
                            at_ps.tile([P, S], f32, name="sps", tag="sps")
                            for _ in range(2)
                        ]
                        for qc in range(QC):
                            for sub in range(2):
                                off = sub * DK
                                nc.tensor.matmul(
                                    spss[sub][:, qc * 512 : (qc + 1) * 512],
                                    kt_[t][off : off + DK, kt * P : (kt + 1) * P],
                                    qt[t][off : off + DK, qc * 512 : (qc + 1) * 512],
                                    start=True,
                                    stop=True,
                                    tile_position=(off, 0),
                                )
                        for sub in range(2):
                            nc.scalar.activation(
                                pts[sub][kt][:],
                                spss[sub][:],
                                EXP,
                                bias=maskb[:, kt : kt + 1],
                                scale=ISCALE,
                            )
                        if drip:
                            drip.pop(0)()
                        if drip:
                            drip.pop(0)()
                    return pts

                # ---- schedule ----
                def proj_tasks(o):
                    ts = []
                    for c0 in range(0, S, 512):
                        ts.append(lambda o=o, c0=c0: qk_chain(wq, xqT, bq_t, qt, o, c0, 512))
                    for c0 in range(0, sk, 512):
                        ts.append(lambda o=o, c0=c0: qk_chain(wk, xkT, bk_t, kt_, o, c0, min(512, sk - c0)))
                    return ts

                for task in proj_tasks(0):
                    task()

                drip = [lambda: bias_broadcast(bv_b, bv_t)]
                drip += [lambda rt=rt: v_chain(rt) for rt in range(KC)]
                drip += proj_tasks(1)
                drip += [lambda: bias_broadcast(bo_b, bo_t)]
                pts_prev = scores_window(0, drip)
                for task in drip:
                    task()

                for t in range(1, HP):
                    drip = list(pv_and_norm_tasks(t - 1, pts_prev))
                    if t + 1 < HP:
                        drip += proj_tasks(t + 1)
                    pts_prev = scores_window(t, drip)
                    for task in drip:
                        task()

                # tail: free score banks, interleave pair 3's PV/norm with
                # the output projection (rt 0-3 need at cols 0:512 = qc 0)
                sps_ctx.close()
                with ExitStack() as cctx:
                    y_pool = cctx.enter_context(tc.tile_pool(name="y", bufs=2))
                    y_ps = cctx.enter_context(
                        tc.tile_pool(name="ypsum", bufs=2, space="PSUM")
                    )

                    def yo_chain(rt):
                        yps = y_ps.tile([P, D], f32, name="yps", tag="yps")
                        for t in range(HP):
                            nc.tensor.matmul(
                                yps[:],
                                at[t][:, rt * P : (rt + 1) * P],
                                wo[t][:],
                                start=(t == 0),
                                stop=(t == HP - 1),
                            )
                        yt = y_pool.tile([P, D], f32, name="yt", tag="yt")
                        nc.vector.tensor_add(yt[:], yps[:], bo_b[:])
                        nc.sync.dma_start(y_d[rt], yt[:])

                    tail = list(pv_and_norm_tasks(HP - 1, pts_prev))
                    for task in tail[0:3]:
                        task()
                    for rt in range(0, 2):
                        yo_chain(rt)
                    for task in tail[3:5]:
                        task()
                    for rt in range(2, 4):
                        yo_chain(rt)
                    tail[5]()
                    for rt in range(4, RT):
                        yo_chain(rt)

        if loop_reps is None:
            emit()
        else:
            ET = mybir.EngineType
            with tc.For_i(
                0,
                loop_reps,
                1,
                hint_engines=(ET.PE, ET.Activation, ET.DVE, ET.SP, ET.Pool),
            ):
                emit()

    nc.compile()
    return nc


def get_nc(loop_reps=None, sk=SK_DEFAULT):
    key = ("nc", loop_reps, sk)
    if key not in _CACHED:
        _CACHED[key] = _build_nc(loop_reps, sk)
    return _CACHED[key]


def _sk_for_mask(mask):
    counts = np.asarray(mask).reshape(B, S).astype(np.int64).sum(axis=1)
    need = max(int(counts.max()), 1)
    return max(P, int(np.ceil(need / P) * P))


def make_in_maps(query, key, value, mask, Wq, bq, Wk, bk, Wv, bv, Wo, bo):
    """Shard full inputs into per-core input maps (host-side numpy)."""
    import ml_dtypes

    bf = ml_dtypes.bfloat16
    f = np.float32
    query = np.asarray(query, f)
    key = np.asarray(key, f)
    value = np.asarray(value, f)
    mask = np.asarray(mask)
    sk = _sk_for_mask(mask)
    kc = sk // P

    def wtiles(W):
        return np.ascontiguousarray(np.asarray(W, f).reshape(KI, P, D)).astype(bf)

    wq_t, wk_t, wv_t, wo_t = wtiles(Wq), wtiles(Wk), wtiles(Wv), wtiles(Wo)
    bq_t = np.ascontiguousarray(np.asarray(bq, f).reshape(KI, P).T)
    bk_t = np.ascontiguousarray(np.asarray(bk, f).reshape(KI, P).T)
    bv_t = np.ascontiguousarray(np.asarray(bv, f).reshape(1, D)).astype(bf)
    bo_t = np.ascontiguousarray(np.asarray(bo, f).reshape(1, D)).astype(bf)

    in_maps = []
    for c in range(B):
        keep = np.flatnonzero(mask[c, 0] != 0)
        nk = len(keep)
        kc_pad = np.zeros((sk, D), f)
        kc_pad[:nk] = key[c][keep]
        vc_pad = np.zeros((sk, D), f)
        vc_pad[:nk] = value[c][keep]
        mb = np.full(sk, f(MASK_NEG), f)
        mb[:nk] = 0.0
        mb = np.ascontiguousarray(mb.reshape(kc, P).T)

        xqT = np.ascontiguousarray(query[c].T).reshape(KI, P, S).astype(bf)
        xkT = np.ascontiguousarray(kc_pad.T).reshape(KI, P, sk).astype(bf)
        xvT = np.ascontiguousarray(vc_pad.T).reshape(KI, P, sk).astype(bf)
        in_maps.append(
            {
                "xqT": xqT,
                "xkT": xkT,
                "xvT": xvT,
                "maskb": mb,
                "wq": wq_t,
                "wk": wk_t,
                "wv": wv_t,
                "wo": wo_t,
                "bq": bq_t,
                "bk": bk_t,
                "bv": bv_t,
                "bo": bo_t,
            }
        )
    return in_maps, sk


def kernel(**inputs):
    from concourse.bass_utils import run_bass_kernel_spmd

    in_maps, sk = make_in_maps(**inputs)
    nc = get_nc(sk=sk)
    res = run_bass_kernel_spmd(nc, in_maps, core_ids=list(range(B)))
    out = np.stack([res.results[c]["y"].reshape(S, D) for c in range(B)])
    return out.astype(np.float32)


# revision 32
# speedup vs baseline: 3.3364x; 1.0323x over previous
"""MultiHeadedAttention Trainium2 Bass kernel (v4: deep pipelining).

Reference (per batch element b, full shapes B=8, S=1024, D=512, H=8, DK=64):
    Q = x_q @ Wq + bq ; K = x_k @ Wk + bk ; V = x_v @ Wv + bv   (per-head split)
    S = Q K^T / sqrt(DK);  S masked where mask==0 -> -inf
    P = softmax(S); P zeroed where mask==0
    Y = (P V, heads concat) @ Wo + bo

Sharding: pure data parallel over batch — core c computes batch element c.

Host prep: transpose x inputs, cast to bf16, COMPACT keys/values by the
mask (masked keys have exactly-zero probs; attention is drop-invariant),
pad to SK=ceil(max_count/128)*128 with maskbias -30000 (exp -> 0).

Engine plan (per core, SK=640):
  ACT: 40x Exp[128,1024] ~42us — the bottleneck stream; never used for DMA.
  PE:  projections (4-chains), 64-row head-pair score matmuls (row groups
       0/64 run concurrently at 2 bf16 rows/cyc), PV 65-row chains (ones
       column in v_aug gives the softmax denominator for free), out-proj.
  DVE: psum drains (+bias, bf16 cast), reciprocals, normalization muls.
  Pool(gpsimd): DMA queue + partition_broadcast of reciprocal rows.
  Scheduling: pair t's PV/norm and pair t+1's projections DRIP between
  pair t+1's score matmuls so ACT never starves; input/weight/persist
  tiles are double-buffered so iteration i+1's DMA and projections overlap
  iteration i's attention tail (the For_i loop pipelines across
  iterations — engines sync only on data).
PSUM: scores 2x[128,1024] (4 banks) + shared [128,512] work pool x4
      (PV accumulators, projection chains, out-proj) = 8 banks.
"""

import numpy as np

B, S, D, H = 8, 1024, 512, 8
DK = D // H  # 64
P = 128
KI = D // P  # 4 in-feature tiles
RT = S // P  # 8 q row tiles
QC = S // 512  # 2 q chunks of 512
HP = H // 2  # 4 head pairs
SK_DEFAULT = 640
MASK_NEG = -30000.0  # exp(-30000) == 0.0 in f32

_CACHED = {}


def _build_nc(loop_reps=None, sk=SK_DEFAULT):
    import concourse.mybir as mybir
    import concourse.tile as tile
    from concourse import bacc

    f32 = mybir.dt.float32
    f32r = mybir.dt.float32r
    bf16 = mybir.dt.bfloat16
    EXP = mybir.ActivationFunctionType.Exp
    ISCALE = 1.0 / float(np.sqrt(DK))
    KC = sk // P  # kv chunks of 128

    nc = bacc.Bacc("TRN2")

    xqT_d = nc.dram_tensor("xqT", (KI, P, S), bf16, kind="ExternalInput")
    xkT_d = nc.dram_tensor("xkT", (KI, P, sk), bf16, kind="ExternalInput")
    xvT_d = nc.dram_tensor("xvT", (KI, P, sk), bf16, kind="ExternalInput")
    maskb_d = nc.dram_tensor("maskb", (P, KC), f32, kind="ExternalInput")
    wq_d = nc.dram_tensor("wq", (KI, P, D), bf16, kind="ExternalInput")
    wk_d = nc.dram_tensor("wk", (KI, P, D), bf16, kind="ExternalInput")
    wv_d = nc.dram_tensor("wv", (KI, P, D), bf16, kind="ExternalInput")
    wo_d = nc.dram_tensor("wo", (KI, P, D), bf16, kind="ExternalInput")
    bq_d = nc.dram_tensor("bq", (P, KI), f32, kind="ExternalInput")
    bk_d = nc.dram_tensor("bk", (P, KI), f32, kind="ExternalInput")
    bv_d = nc.dram_tensor("bv", (1, D), bf16, kind="ExternalInput")
    bo_d = nc.dram_tensor("bo", (1, D), bf16, kind="ExternalInput")
    y_d = nc.dram_tensor("y", (RT, P, D), f32, kind="ExternalOutput")

    with tile.TileContext(nc) as tc, nc.allow_low_precision(
        reason="bf16 matmuls with fp32 PSUM accumulation; tolerance is 2e-2"
    ):
        from contextlib import ExitStack

        def emit():
            with ExitStack() as ctx:
                # bufs=2 on inputs/persist: iteration i+1's DMA + projections
                # overlap iteration i's attention tail
                const = ctx.enter_context(tc.tile_pool(name="const", bufs=1))
                persist = ctx.enter_context(tc.tile_pool(name="persist", bufs=1))
                one_p = ctx.enter_context(tc.tile_pool(name="onep", bufs=1))
                pt_pool = ctx.enter_context(tc.tile_pool(name="pt", bufs=20))
                rec_pool = ctx.enter_context(tc.tile_pool(name="rec", bufs=2))
                work_ps = ctx.enter_context(
                    tc.tile_pool(name="work", bufs=4, space="PSUM")
                )
                sps_ctx = ctx.enter_context(ExitStack())
                at_ps = sps_ctx.enter_context(
                    tc.tile_pool(name="spsum", bufs=2, space="PSUM")
                )

                wq = [const.tile([P, D], bf16, name=f"wq{i}", tag=f"wq{i}") for i in range(KI)]
                wk = [const.tile([P, D], bf16, name=f"wk{i}", tag=f"wk{i}") for i in range(KI)]
                wv = [const.tile([P, D], bf16, name=f"wv{i}", tag=f"wv{i}") for i in range(KI)]
                wo = [const.tile([P, D], bf16, name=f"wo{i}", tag=f"wo{i}") for i in range(KI)]
                bq_t = const.tile([P, KI], f32, name="bq_t", tag="bq")
                bk_t = const.tile([P, KI], f32, name="bk_t", tag="bk")
                bv_t = const.tile([1, D], bf16, name="bv_t", tag="bv")
                bo_t = const.tile([1, D], bf16, name="bo_t", tag="bo")
                maskb = const.tile([P, KC], f32, name="maskb", tag="maskb")
                xqT = [const.tile([P, S], bf16, name=f"xq{i}", tag=f"xq{i}") for i in range(KI)]
                xkT = [const.tile([P, sk], bf16, name=f"xk{i}", tag=f"xk{i}") for i in range(KI)]
                xvT = [const.tile([P, sk], bf16, name=f"xv{i}", tag=f"xv{i}") for i in range(KI)]

                ones_t = one_p.tile([1, P], bf16, name="ones_t", tag="ones")
                nc.vector.memset(ones_t[:].bitcast(bf16), 1.0)
                # bias broadcast rows (built once per iteration via matmul)
                bv_b = persist.tile([P, D], f32, name="bv_b", tag="bvb")
                bo_b = persist.tile([P, D], f32, name="bo_b", tag="bob")

                qt = [persist.tile([P, S], bf16, name=f"qt{i}", tag=f"qt{i}") for i in range(KI)]
                kt_ = [persist.tile([P, sk], bf16, name=f"kt{i}", tag=f"kt{i}") for i in range(KI)]
                v_aug = [persist.tile([P, H, DK + 1], bf16, name=f"va{i}", tag=f"va{i}") for i in range(KC)]
                at = [persist.tile([P, S], bf16, name=f"at{i}", tag=f"at{i}") for i in range(HP)]

                # DMA: consumption order, balanced across the three HWDGE
                # queues (SP / gpsimd / ACT — ACT only issues before its exp
                # stream starts). y output DMA rides SP at the end.
                for i in range(2):
                    nc.sync.dma_start(wq[i][:], wq_d[i])
                    nc.sync.dma_start(xqT[i][:], xqT_d[i])
                    nc.scalar.dma_start(wq[i + 2][:], wq_d[i + 2])
                    nc.scalar.dma_start(xqT[i + 2][:], xqT_d[i + 2])
                    nc.gpsimd.dma_start(wk[i][:], wk_d[i])
                    nc.gpsimd.dma_start(xkT[i][:], xkT_d[i])
                nc.scalar.dma_start(wk[2][:], wk_d[2])
                nc.scalar.dma_start(xkT[2][:], xkT_d[2])
                nc.gpsimd.dma_start(wk[3][:], wk_d[3])
                nc.gpsimd.dma_start(xkT[3][:], xkT_d[3])
                nc.sync.dma_start(bq_t[:], bq_d[:])
                nc.gpsimd.dma_start(bk_t[:], bk_d[:])
                nc.gpsimd.dma_start(maskb[:], maskb_d[:])
                for i in range(2):
                    nc.scalar.dma_start(wv[i][:], wv_d[i])
                    nc.scalar.dma_start(xvT[i][:], xvT_d[i])
                    nc.gpsimd.dma_start(wv[i + 2][:], wv_d[i + 2])
                    nc.gpsimd.dma_start(xvT[i + 2][:], xvT_d[i + 2])
                nc.scalar.dma_start(bv_t[:], bv_d[:])
                for i in range(KI):
                    nc.gpsimd.dma_start(wo[i][:], wo_d[i])
                nc.gpsimd.dma_start(bo_t[:], bo_d[:])

                def bias_broadcast(dst, src):
                    ps = work_ps.tile([P, 512], f32, name="wps", tag="wps")
                    nc.tensor.matmul(
                        ps[:], ones_t[0:1, :], src[0:1, :], start=True, stop=True
                    )
                    nc.vector.tensor_copy(dst[:], ps[:])

                def qk_chain(w, x, bias, dst, o, c0, cw):
                    ps = work_ps.tile([P, 512], f32, name="wps", tag="wps")
                    for ki in range(KI):
                        nc.tensor.matmul(
                            ps[:, 0:cw],
                            w[ki][:, o * P : (o + 1) * P],
                            x[ki][:, c0 : c0 + cw],
                            start=(ki == 0),
                            stop=(ki == KI - 1),
                        )
                    nc.vector.tensor_scalar_add(
                        dst[o][:, c0 : c0 + cw], ps[:, 0:cw], bias[:, o : o + 1]
                    )

                def v_chain(rt):
                    ps = work_ps.tile([P, 512], f32, name="wps", tag="wps")
                    for ki in range(KI):
                        nc.tensor.matmul(
                            ps[:],
                            xvT[ki][:, rt * P : (rt + 1) * P],
                            wv[ki][:],
                            start=(ki == 0),
                            stop=(ki == KI - 1),
                        )
                    nc.vector.tensor_add(
                        v_aug[rt][:, :, 0:DK],
                        ps[:].rearrange("p (h d) -> p h d", h=H),
                        bv_b[:].rearrange("p (h d) -> p h d", h=H),
                    )
                    nc.gpsimd.memset(v_aug[rt][:, :, DK : DK + 1].bitcast(bf16), 1.0)

                def pv_and_norm_tasks(t, pts):
                    """Deferred PV + normalization for pair t as drip units.
                    All four PV accumulators are independent (work pool);
                    normalization hangs off DVE/Pool without gating PE."""
                    for qc in range(QC):
                        opss = []
                        for sub in range(2):
                            ops = work_ps.tile([P, 512], f32, name="wps", tag="wps")
                            opss.append(ops)

                            def pv(ops=ops, sub=sub, qc=qc):
                                for kt in range(KC):
                                    nc.tensor.matmul(
                                        ops[0 : DK + 1, :],
                                        v_aug[kt][:, 2 * t + sub, 0 : DK + 1],
                                        pts[sub][kt][:, qc * 512 : (qc + 1) * 512],
                                        start=(kt == 0),
                                        stop=(kt == KC - 1),
                                    )

                            yield pv

                        def norm(opss=opss, qc=qc):
                            # HW partition_broadcast only writes/reads from
                            # partition base 0 — separate tiles per sub
                            for sub in range(2):
                                rec = rec_pool.tile(
                                    [1, 512], f32, name="rec", tag=f"rec{sub}"
                                )
                                rbs = rec_pool.tile(
                                    [DK, 512], f32, name="rbs", tag=f"rbs{sub}"
                                )
                                nc.vector.reciprocal(
                                    rec[0:1, :], opss[sub][DK : DK + 1, :]
                                )
                                nc.gpsimd.partition_broadcast(
                                    rbs[:], rec[0:1, :]
                                )
                                off = sub * DK
                                nc.vector.tensor_mul(
                                    at[t][off : off + DK, qc * 512 : (qc + 1) * 512],
                                    opss[sub][0:DK, :],
                                    rbs[:],
                                )

                        yield norm

                def scores_window(t, drip):
                    """Pair t's scores+exp; drip deferred tasks between
                    kv-chunks so PE work fills ACT's exp time."""
                    pts = [
                        [pt_pool.tile([P, S], bf16, name="pt", tag="pt") for _ in range(KC)]
                        for _ in range(2)
                    ]
                    for kt in range(KC):
                        spss = [
                            at_ps.tile([P, S], f32, name="sps", tag="sps")
                            for _ in range(2)
                        ]
                        for qc in range(QC):
                            for sub in range(2):
                                off = sub * DK
                                nc.tensor.matmul(
                                    spss[sub][:, qc * 512 : (qc + 1) * 512],
                                    kt_[t][off : off + DK, kt * P : (kt + 1) * P],
                                    qt[t][off : off + DK, qc * 512 : (qc + 1) * 512],
                                    start=True,
                                    stop=True,
                                    tile_position=(off, 0),
                                )
                        for sub in range(2):
                            nc.scalar.activation(
                                pts[sub][kt][:],
                                spss[sub][:],
                                EXP,
                                bias=maskb[:, kt : kt + 1],
                                scale=ISCALE,
                            )
                        if drip:
                            drip.pop(0)()
                        if drip:
                            drip.pop(0)()
                    return pts

                # ---- schedule ----
                def proj_tasks(o):
                    ts = []
                    for c0 in range(0, S, 512):
                        ts.append(lambda o=o, c0=c0: qk_chain(wq, xqT, bq_t, qt, o, c0, 512))
                    for c0 in range(0, sk, 512):
                        ts.append(lambda o=o, c0=c0: qk_chain(wk, xkT, bk_t, kt_, o, c0, min(512, sk - c0)))
                    return ts

                for task in proj_tasks(0):
                    task()

                drip = [lambda: bias_broadcast(bv_b, bv_t)]
                drip += [lambda rt=rt: v_chain(rt) for rt in range(KC)]
                drip += proj_tasks(1)
                drip += [lambda: bias_broadcast(bo_b, bo_t)]
                pts_prev = scores_window(0, drip)
                for task in drip:
                    task()

                for t in range(1, HP):
                    drip = list(pv_and_norm_tasks(t - 1, pts_prev))
                    if t + 1 < HP:
                        drip += proj_tasks(t + 1)
                    pts_prev = scores_window(t, drip)
                    for task in drip:
                        task()

                # tail: free score banks, interleave pair 3's PV/norm with
                # the output projection (rt 0-3 need at cols 0:512 = qc 0)
                sps_ctx.close()
                with ExitStack() as cctx:
                    y_pool = cctx.enter_context(tc.tile_pool(name="y", bufs=2))
                    y_ps = cctx.enter_context(
                        tc.tile_pool(name="ypsum", bufs=2, space="PSUM")
                    )

                    def yo_chain(rt):
                        yps = y_ps.tile([P, D], f32, name="yps", tag="yps")
                        for t in range(HP):
                            nc.tensor.matmul(
                                yps[:],
                                at[t][:, rt * P : (rt + 1) * P],
                                wo[t][:],
                                start=(t == 0),
                                stop=(t == HP - 1),
                            )
                        yt = y_pool.tile([P, D], f32, name="yt", tag="yt")
                        nc.vector.tensor_add(yt[:], yps[:], bo_b[:])
                        # spread output DMA over all three queues (2MB f32
                        # on one ~150GB/s queue would be a ~13us tail)
                        eng = (nc.sync, nc.scalar, nc.gpsimd)[rt % 3]
                        eng.dma_start(y_d[rt], yt[:])

                    tail = list(pv_and_norm_tasks(HP - 1, pts_prev))
                    for task in tail[0:3]:
                        task()
                    for rt in range(0, 2):
                        yo_chain(rt)
                    for task in tail[3:5]:
                        task()
                    for rt in range(2, 4):
                        yo_chain(rt)
                    tail[5]()
                    for rt in range(4, RT):
                        yo_chain(rt)

        if loop_reps is None:
            emit()
        else:
            ET = mybir.EngineType
            with tc.For_i(
                0,
                loop_reps,
                1,
                hint_engines=(ET.PE, ET.Activation, ET.DVE, ET.SP, ET.Pool),
            ):
                emit()

    nc.compile()
    return nc


def get_nc(loop_reps=None, sk=SK_DEFAULT):
    key = ("nc", loop_reps, sk)
    if key not in _CACHED:
        _CACHED[key] = _build_nc(loop_reps, sk)
    return _CACHED[key]


def _sk_for_mask(mask):
    counts = np.asarray(mask).reshape(B, S).astype(np.int64).sum(axis=1)
    need = max(int(counts.max()), 1)
    return max(P, int(np.ceil(need / P) * P))


def make_in_maps(query, key, value, mask, Wq, bq, Wk, bk, Wv, bv, Wo, bo):
    """Shard full inputs into per-core input maps (host-side numpy)."""
    import ml_dtypes

    bf = ml_dtypes.bfloat16
    f = np.float32
    query = np.asarray(query, f)
    key = np.asarray(key, f)
    value = np.asarray(value, f)
    mask = np.asarray(mask)
    sk = _sk_for_mask(mask)
    kc = sk // P

    def wtiles(W):
        return np.ascontiguousarray(np.asarray(W, f).reshape(KI, P, D)).astype(bf)

    wq_t, wk_t, wv_t, wo_t = wtiles(Wq), wtiles(Wk), wtiles(Wv), wtiles(Wo)
    bq_t = np.ascontiguousarray(np.asarray(bq, f).reshape(KI, P).T)
    bk_t = np.ascontiguousarray(np.asarray(bk, f).reshape(KI, P).T)
    bv_t = np.ascontiguousarray(np.asarray(bv, f).reshape(1, D)).astype(bf)
    bo_t = np.ascontiguousarray(np.asarray(bo, f).reshape(1, D)).astype(bf)

    in_maps = []
    for c in range(B):
        keep = np.flatnonzero(mask[c, 0] != 0)
        nk = len(keep)
        kc_pad = np.zeros((sk, D), f)
        kc_pad[:nk] = key[c][keep]
        vc_pad = np.zeros((sk, D), f)
        vc_pad[:nk] = value[c][keep]
        mb = np.full(sk, f(MASK_NEG), f)
        mb[:nk] = 0.0
        mb = np.ascontiguousarray(mb.reshape(kc, P).T)

        xqT = np.ascontiguousarray(query[c].T).reshape(KI, P, S).astype(bf)
        xkT = np.ascontiguousarray(kc_pad.T).reshape(KI, P, sk).astype(bf)
        xvT = np.ascontiguousarray(vc_pad.T).reshape(KI, P, sk).astype(bf)
        in_maps.append(
            {
                "xqT": xqT,
                "xkT": xkT,
                "xvT": xvT,
                "maskb": mb,
                "wq": wq_t,
                "wk": wk_t,
                "wv": wv_t,
                "wo": wo_t,
                "bq": bq_t,
                "bk": bk_t,
                "bv": bv_t,
                "bo": bo_t,
            }
        )
    return in_maps, sk


def kernel(**inputs):
    from concourse.bass_utils import run_bass_kernel_spmd

    in_maps, sk = make_in_maps(**inputs)
    nc = get_nc(sk=sk)
    res = run_bass_kernel_spmd(nc, in_maps, core_ids=list(range(B)))
    out = np.stack([res.results[c]["y"].reshape(S, D) for c in range(B)])
    return out.astype(np.float32)
